# revision 2
# baseline (speedup 1.0000x reference)
"""Trainium2 Bass kernel for nn_BlockWithAttention — fp8 DoubleRow convs.

Sharding: data-parallel over batch (B=16 -> 2 samples/core x 8 cores).
BN batch stats synced via four tiny HBM AllGathers (one per BN per
128-channel block), pipelined against conv compute.

Convs run as compensated fp8e4m3 DoubleRow matmuls: operands are split
hi/lo (v = hi + lo, both e4m3, per-tensor scaled out of the subnormal
range), and each conv accumulates three term streams hi*hi + hi*lo +
lo*hi in fp32 PSUM (lo*lo dropped, ~0.4% rms).  DoubleRow pairs two
3x3 taps (same partitions, different free offsets) so the ki-split
partial-accumulation pipelining that hides collective latency is kept:
conv2/conv3 contract ki=0 in an open phase, ki=1 closes, with the
center tap paired across ki in the close phase.  conv1 pairs ki per
tap (no BN wait on its input).  Per-tensor scales fold away: BN is
scale-invariant, the relu epilogue descales via the activation `scale`
field, and conv3's bias-add epilogue gains a multiply constant.
Attention stays float32r.

Self-contained: hardcodes shapes; needs concourse + numpy + ml_dtypes.
"""
import numpy as np
import ml_dtypes

import concourse.bass as bass
import concourse.mybir as mybir
from concourse.ap import AP
from concourse.bass_utils import run_bass_kernel_spmd
from concourse.tile import TileContext
from concourse.tile_rust import add_dep_helper

# ---- problem constants ----
B, C, H, W, T, CQ = 16, 256, 32, 32, 256, 32
NCORES = 8
BL = B // NCORES            # samples per core
KT = C // 128               # 128-channel blocks
HP, WP = H + 2, W + 2       # padded image
NPAD = HP * WP              # 1156
NPIX = B * H * W            # BN stat count (full batch)
N = H * W                   # 1024 spatial positions
RH = 16                     # rows per 512-px half
EPS = 1e-5
CWC = 9 * KT * KT * 128     # conv weight columns (4608)

# fp8 per-tensor scales (powers of two: exact in all formats)
A_X = 32.0                  # conv1 input
A_W = 256.0                 # conv weights
A_H = 8.0                   # normalized h1/h2 (folded into bn gamma/beta + te)

F32 = mybir.dt.float32
F32R = mybir.dt.float32r
F16 = mybir.dt.float16
FP8 = mybir.dt.float8e4
AX = mybir.AxisListType
ALU = mybir.AluOpType
AF = mybir.ActivationFunctionType
DR = mybir.MatmulPerfMode.DoubleRow

_wsplit_counter = [0]


def _split_packed_waits(nc, max_waits: int = 1):
    """The walrus build here rejects >1-2 packed sync-waits per instruction
    ("Too many sync wait commands"). Move excess waits onto standalone
    single-wait EventSemaphore carriers inserted before the instruction
    (same engine -> program order preserves gating)."""
    for f in nc.m.functions:
        for bb in f.blocks:
            il = bb.instructions
            i = 0
            while i < len(il):
                inst = il[i]
                si = inst.sync_info
                if si is not None and len(si.on_wait) > max_waits:
                    waits = list(si.on_wait)
                    movable = [w for w in waits if w.wait_reg is None]
                    fixed = [w for w in waits if w.wait_reg is not None]
                    keep_n = max(0, max_waits - len(fixed))
                    kept = fixed + movable[:keep_n]
                    move = movable[keep_n:]
                    if not move:
                        i += 1
                        continue
                    si.on_wait = kept
                    for w in move:
                        _wsplit_counter[0] += 1
                        ev = mybir.InstEventSemaphore(
                            name=f"I-wsplit-{_wsplit_counter[0]}",
                            opcode="EventSemaphore",
                            engine=inst.engine,
                            sync_info=mybir.SyncInfo(on_wait=[w], on_update=[]),
                        )
                        il.insert(i, ev)
                        i += 1
                i += 1


def _pad3(tile):
    """[128, NPAD] pad tile viewed as [128, HP, WP]."""
    return tile[:, :].rearrange("p (r c) -> p r c", c=WP)


def _interior(tile, r0=0, nr=H):
    """interior rows r0..r0+nr of the HxW image inside a pad tile."""
    return _pad3(tile)[:, 1 + r0:1 + r0 + nr, 1:1 + W]


U32 = mybir.dt.uint32
ONE_F32_BITS = 0x3F800000


def _memset_border(nc, tile, ki_pitch=None):
    """zero the 1-px border of a pad tile ([128, NPAD] or [128, 2*NPAD])."""
    nk = 1 if ki_pitch is None else 2
    for k in range(nk):
        v = tile[:, k * NPAD:(k + 1) * NPAD].rearrange("p (r c) -> p r c", c=WP)
        iv = U32 if mybir.dt.size(tile.dtype) == 4 else (
            mybir.dt.uint16 if mybir.dt.size(tile.dtype) == 2 else mybir.dt.uint8)
        nc.gpsimd.memset(v[:, 0:1, :].bitcast(iv), 0)
        nc.gpsimd.memset(v[:, HP - 1:HP, :].bitcast(iv), 0)
        nc.gpsimd.memset(v[:, 1:HP - 1, 0:1].bitcast(iv), 0)
        nc.gpsimd.memset(v[:, 1:HP - 1, WP - 1:WP].bitcast(iv), 0)


# DoubleRow tap pairs within one ki block: (dya, dxa, dyb, dxb)
TAP_PAIRS = [(0, 0, 0, 2), (1, 0, 1, 2), (2, 0, 2, 2), (0, 1, 2, 1)]


def _ap4(base_ap, off, dims):
    """strided view of a tile: dims are [stride, count] pairs for the free
    dims; the partition dim is copied from the tile's own AP."""
    return AP(base_ap.tensor, base_ap.offset + off,
              [list(base_ap.ap[0])] + dims)


def build(warm1: int = 195, warm2: int = 0, warm3: int = 125, warm4: int = 46):
    nc = bass.Bass(num_devices=NCORES)
    dt = F32R

    # ---- DRAM I/O ----
    # x: hi/lo fp8, both ki blocks side by side per sample
    xp_d = nc.dram_tensor("xp", [BL, 2, 128, 2 * NPAD], FP8, kind="ExternalInput")
    # conv weights: [conv(3), hi/lo(2), 128, CWC] fp8
    cw_d = nc.dram_tensor("cw", [3, 2, 128, CWC], FP8, kind="ExternalInput")
    w1t_d = nc.dram_tensor("w1t", [KT, 128, T], F32R, kind="ExternalInput")
    w2t_d = nc.dram_tensor("w2t", [KT, 128, C], F32R, kind="ExternalInput")
    # packed per-channel constants: cols 0-5 conv biases (ci*2+k),
    # 6-9 bn gammas*A_H (i*2+k), 10-13 bn betas*A_H, 14-15 b_t1,
    # 16-17 b_t2*A_H, 18-21 t^T per-core slices (k*BL+s)
    consts_d = nc.dram_tensor("consts", [128, 22], F32R, kind="ExternalInput")
    wqt_d = nc.dram_tensor("wqt", [KT, 128, CQ], dt, kind="ExternalInput")
    wkt_d = nc.dram_tensor("wkt", [KT, 128, CQ], dt, kind="ExternalInput")
    wvt_d = nc.dram_tensor("wvt", [KT, 128, C], dt, kind="ExternalInput")
    bq_d = nc.dram_tensor("bq", [CQ, 1], F32R, kind="ExternalInput")
    bk_d = nc.dram_tensor("bk", [CQ, 1], F32R, kind="ExternalInput")
    bvbc_d = nc.dram_tensor("bvbc", [128, C], dt, kind="ExternalInput")
    out_d = nc.dram_tensor("out", [BL, KT, 128, N], F32R, kind="ExternalOutput")

    # collective bounce buffers (HBM-HBM): one merged for BN1 (both ko
    # blocks), one per ko for BN2
    ccm_in = nc.dram_tensor("ccm_in", [128, 4], F32)
    ccm_out = nc.dram_tensor("ccm_out", [NCORES, 128, 4], F32,
                             addr_space="Shared")
    cc_in = [nc.dram_tensor(f"cc{i}_in", [128, 2], F32) for i in range(2)]
    cc_out = [nc.dram_tensor(f"cc{i}_out", [NCORES, 128, 2], F32,
                             addr_space="Shared") for i in range(2)]

    with TileContext(nc) as tc:
        with (
            tc.tile_pool(name="pconst", bufs=1) as pc,
            tc.tile_pool(name="pcw", bufs=6) as pcw,
            tc.tile_pool(name="ppad", bufs=8) as ppad,
            tc.tile_pool(name="pquant", bufs=8) as pq,
            tc.tile_pool(name="py", bufs=4) as py,
            tc.tile_pool(name="psq", bufs=2) as psq,
            tc.tile_pool(name="pattn", bufs=1) as pat,
            tc.tile_pool(name="pstats", bufs=1) as pst,
            tc.tile_pool(name="ppsum", bufs=8, space="PSUM") as pps,
        ):
            def psum(nm):
                return pps.tile([128, 512], F32, tag="ps", name=nm)

            # ---- SBUF tiles ----
            # conv weights, hi/lo fp8
            cw_sb = [[pcw.tile([128, CWC], FP8, tag="cw", name=f"cw{ci}{t}")
                      for t in range(2)] for ci in range(3)]
            # conv1 input: [s][t] combined-ki fp8 pads (host-prepared)
            x8 = [[ppad.tile([128, 2 * NPAD], FP8, tag="padx", bufs=4,
                             name=f"x8{s}{t}") for t in range(2)]
                  for s in range(BL)]
            # h staging pads (fp16, epilogue + in-place normalize)
            h1_pad = [[ppad.tile([128, NPAD], F16, tag="pad", bufs=8,
                                 name=f"h1p{s}{k}") for k in range(KT)]
                      for s in range(BL)]
            h2_pad = [[ppad.tile([128, NPAD], F16, tag="pad", bufs=8,
                                 name=f"h2p{s}{k}") for k in range(KT)]
                      for s in range(BL)]
            # quantized hi/lo pads, both ki combined: [s][t]
            h1q = [[pq.tile([128, 2 * NPAD], FP8, tag="q", name=f"h1q{s}{t}")
                    for t in range(2)] for s in range(BL)]
            h2q = [[pq.tile([128, 2 * NPAD], FP8, tag="q", name=f"h2q{s}{t}")
                    for t in range(2)] for s in range(BL)]

            w1t_sb = [pc.tile([128, T], F32R, name=f"w1t{k}") for k in range(KT)]
            w2t_sb = [pc.tile([128, C], F32R, name=f"w2t{k}") for k in range(KT)]
            consts_sb = pc.tile([128, 22], F32R, name="consts_sb")

            def ccol(j, n=1):
                return consts_sb[:, j:j + n]

            cb_sb = [[ccol(ci * KT + k) for k in range(KT)] for ci in range(3)]
            bng_sb = [[ccol(6 + i * KT + k) for k in range(KT)] for i in range(2)]
            bnb_sb = [[ccol(10 + i * KT + k) for k in range(KT)] for i in range(2)]
            bt1_sb = [ccol(14 + k) for k in range(KT)]
            bt2_sb = [ccol(16 + k) for k in range(KT)]
            tt_sb = [ccol(18 + k * BL, BL) for k in range(KT)]
            wqt_sb = [pc.tile([128, CQ], dt, name=f"wqt{k}") for k in range(KT)]
            wkt_sb = [pc.tile([128, CQ], dt, name=f"wkt{k}") for k in range(KT)]
            wvt_sb = [pc.tile([128, C], dt, name=f"wvt{k}") for k in range(KT)]
            bq_sb = pc.tile([CQ, 1], F32R, name="bq_sb")
            bk_sb = pc.tile([CQ, 1], F32R, name="bk_sb")
            bvbc_sb = pc.tile([128, C], dt, name="bvbc_sb")
            ones_col = pc.tile([128, 1], dt, name="ones_col")
            ones_row = pc.tile([1, 128], dt, name="ones_row")

            # stats: cols [0:8]=sum(ko,s,half), [8:16]=sumsq(ko,s,half)
            stats = [pst.tile([128, 16], F32, name=f"stats{i}") for i in range(2)]
            ccpm = pst.tile([128, 4], F32, name="ccpm")
            gallm = pst.tile([128, 4 * NCORES], F32, name="gallm")
            ccp = [pst.tile([128, 2], F32, name=f"ccp{i}") for i in range(2)]
            gall = [pst.tile([128, 2 * NCORES], F32, name=f"gall{i}")
                    for i in range(2)]
            glob = [pst.tile([128, 2], F32, name=f"glob{i}") for i in range(4)]

            # =============== DMA schedule ===============
            # sync (HWDGE) queue: conv1 weights in consumption order, then x
            # remainders, then conv2/3 weights.  scalar/gpsimd queues carry
            # the first x tiles so the startup transfers pipeline.
            def cwdma(q, ci, t, c0, c1):
                q.dma_start(out=cw_sb[ci][t][:, c0:c1], in_=cw_d[ci, t, :, c0:c1])

            # conv1 layout is ko-major: ko0 weights = cols [0:2304].
            # First-group pieces go first and small: weights taps 0-2 (hi,
            # lo) on SP; x8 s0 top rows (both ki) on the scalar (hi) and
            # SWDGE (lo) queues.
            def xdma(q, s, t, c0, c1):
                q.dma_start(out=x8[s][t][:, c0:c1], in_=xp_d[s, t, :, c0:c1])

            TOP = 18 * WP
            cwdma(nc.sync, 0, 0, 0, 768)
            xdma(nc.scalar, 0, 0, 0, TOP)
            xdma(nc.gpsimd, 0, 1, 0, TOP)
            cwdma(nc.sync, 0, 1, 0, 768)
            xdma(nc.scalar, 0, 0, NPAD, NPAD + TOP)
            xdma(nc.gpsimd, 0, 1, NPAD, NPAD + TOP)
            cwdma(nc.sync, 0, 0, 768, 2304)
            cwdma(nc.sync, 0, 1, 768, 2304)
            xdma(nc.scalar, 0, 0, TOP, NPAD)
            xdma(nc.gpsimd, 0, 1, TOP, NPAD)
            xdma(nc.scalar, 0, 0, NPAD + TOP, 2 * NPAD)
            xdma(nc.gpsimd, 0, 1, NPAD + TOP, 2 * NPAD)
            cwdma(nc.sync, 0, 0, 2304, CWC)
            cwdma(nc.sync, 0, 1, 2304, CWC)
            xdma(nc.scalar, 1, 0, 0, 2 * NPAD)
            xdma(nc.gpsimd, 1, 1, 0, 2 * NPAD)
            for t in range(2):
                cwdma(nc.sync, 1, t, 0, CWC)
            for t in range(2):
                cwdma(nc.sync, 2, t, 0, CWC)

            # gpsimd (SWDGE) queue: small constants; consts first (conv1
            # epilogue biases need it early)
            nc.gpsimd.dma_start(out=consts_sb[:, :], in_=consts_d[:, :])
            nc.gpsimd.memset(ones_col[:, :].bitcast(U32), ONE_F32_BITS)
            nc.gpsimd.memset(ones_row[:, :].bitcast(U32), ONE_F32_BITS)
            for s in range(BL):
                for k in range(KT):
                    _memset_border(nc, h1_pad[s][k])
                    _memset_border(nc, h2_pad[s][k])
            for k in range(KT):
                nc.gpsimd.dma_start(out=w1t_sb[k][:, :], in_=w1t_d[k, :, :])
                nc.gpsimd.dma_start(out=w2t_sb[k][:, :], in_=w2t_d[k, :, :])
            for k in range(KT):
                nc.gpsimd.dma_start(out=wqt_sb[k][:, :], in_=wqt_d[k, :, :])
                nc.gpsimd.dma_start(out=wkt_sb[k][:, :], in_=wkt_d[k, :, :])
                nc.gpsimd.dma_start(out=wvt_sb[k][:, :], in_=wvt_d[k, :, :])
            nc.gpsimd.dma_start(out=bq_sb[:, :], in_=bq_d[:, :])
            nc.gpsimd.dma_start(out=bk_sb[:, :], in_=bk_d[:, :])
            nc.gpsimd.dma_start(out=bvbc_sb[:, :], in_=bvbc_d[:, :])

            # =============== helpers ===============
            def stat_col(ko, s, half):
                return ko * 4 + s * 2 + half

            def epilogue_bn(bn, h_pads, s, ko, half, ps3, descale):
                """relu+descale+bias (+sum accum) on ACT; sumsq on DVE."""
                c = stat_col(ko, s, half)
                r0 = half * RH
                nc.scalar.activation(
                    _interior(h_pads[s][ko], r0, RH), ps3, AF.Relu,
                    bias=cb_sb[bn][ko][:, :], scale=descale,
                    accum_out=stats[bn][:, c:c + 1],
                )
                sq = psq.tile([128, 512], F32, tag="sq", bufs=1,
                              name=f"sq{bn}_{s}{ko}{half}")
                with nc.allow_low_precision(reason="f32r==f32 bit layout"):
                    nc.vector.scalar_tensor_tensor(
                        out=sq[:, :].rearrange("p (r c) -> p r c", c=W),
                        in0=_interior(h_pads[s][ko], r0, RH),
                        scalar=1.0,
                        in1=_interior(h_pads[s][ko], r0, RH),
                        op0=ALU.bypass, op1=ALU.mult,
                        accum_out=stats[bn][:, 8 + c:9 + c],
                    )

            def cc_launch_bn1():
                """merged BN1 sync: both ko blocks in one AllGather."""
                for ko in range(KT):
                    nc.vector.reduce_sum(ccpm[:, 2 * ko:2 * ko + 1],
                                         stats[0][:, ko * 4:ko * 4 + 4],
                                         axis=AX.X)
                    nc.vector.reduce_sum(ccpm[:, 2 * ko + 1:2 * ko + 2],
                                         stats[0][:, 8 + ko * 4:12 + ko * 4],
                                         axis=AX.X)
                d1 = nc.scalar.dma_start(out=ccm_in[:, :], in_=ccpm[:, :])
                cc = nc.gpsimd.collective_compute(
                    "AllGather", ALU.bypass,
                    replica_groups=[list(range(NCORES))],
                    ins=[ccm_in[:].opt()], outs=[ccm_out[:].opt()],
                )
                add_dep_helper(cc.ins, d1.ins, reason="cc waits on stats dma")
                return cc

            def cc_readback_bn1(cc):
                d2 = nc.sync.dma_start(
                    out=gallm[:, :],
                    in_=ccm_out[:, :, :].rearrange("c p k -> p c k"))
                add_dep_helper(d2.ins, cc.ins, reason="readback waits on cc")

            def cc_launch(bn, ko):
                """BN2 per-ko sync: reduce (DVE) -> HBM -> AllGather."""
                i = ko
                nc.vector.reduce_sum(ccp[i][:, 0:1],
                                     stats[bn][:, ko * 4:ko * 4 + 4], axis=AX.X)
                nc.vector.reduce_sum(ccp[i][:, 1:2],
                                     stats[bn][:, 8 + ko * 4:12 + ko * 4], axis=AX.X)
                d1 = nc.scalar.dma_start(out=cc_in[i][:, :], in_=ccp[i][:, :])
                cc = nc.gpsimd.collective_compute(
                    "AllGather", ALU.bypass,
                    replica_groups=[list(range(NCORES))],
                    ins=[cc_in[i][:].opt()], outs=[cc_out[i][:].opt()],
                )
                add_dep_helper(cc.ins, d1.ins, reason="cc waits on stats dma")
                return cc

            def cc_readback(i, cc):
                # SP queue: idle after the startup weight loads, so the
                # blocking wait-on-cc does not stall any engine sequencer
                d2 = nc.sync.dma_start(
                    out=gall[i][:, :],
                    in_=cc_out[i][:, :, :].rearrange("c p k -> p c k"))
                add_dep_helper(d2.ins, cc.ins, reason="readback waits on cc")

            def warmup(n, ps):
                """Discarded DR matmuls that keep the PE clock ramped through
                a stat-sync bubble; the next real start=True matmul resets the
                bank."""
                lhsT = cw_sb[1][0][:, 0:256].rearrange("p (b m) -> p b m", b=2)
                rhs = cw_sb[1][0][:, 0:1024].rearrange("p (b f) -> p b f", b=2)
                for _ in range(n):
                    nc.tensor.matmul(ps[:, 0:512], lhsT, rhs, start=False,
                                     stop=False, perf_mode=DR,
                                     skip_group_check=True)

            scl = [[None] * KT for _ in range(2)]   # per (bn, ko) [128,1]
            shf = [[None] * KT for _ in range(2)]
            bsk = [[None] * KT for _ in range(BL)]  # bn0 shift + te, per (s, ko)

            def bn_consts(bn, ko):
                """global stat reduce + scale/shift consts, all on DVE."""
                i = bn * 2 + ko
                nc.vector.reduce_sum(
                    glob[i][:, :],
                    gall[i][:, :].rearrange("p (c k) -> p k c", k=2), axis=AX.X)
                mneg = pst.tile([128, 1], F32, name=f"mneg{i}")
                qh = pst.tile([128, 1], F32, name=f"qh{i}")
                var = pst.tile([128, 1], F32, name=f"var{i}")
                rv = pst.tile([128, 1], F32, name=f"rv{i}")
                sc = pst.tile([128, 1], F32, name=f"scl{i}")
                sh = pst.tile([128, 1], F32, name=f"shf{i}")
                nc.vector.tensor_scalar_mul(mneg[:, :], glob[i][:, 0:1], -1.0 / NPIX)
                nc.vector.tensor_scalar(out=qh[:, :], in0=glob[i][:, 1:2],
                                        scalar1=1.0 / NPIX, scalar2=EPS,
                                        op0=ALU.mult, op1=ALU.add)
                t1 = pst.tile([128, 1], F32, name=f"nr1_{i}")
                nc.vector.tensor_tensor(t1[:, :], mneg[:, :], mneg[:, :], ALU.mult)
                nc.vector.tensor_tensor(var[:, :], qh[:, :], t1[:, :], ALU.subtract)
                nc.vector.reciprocal(rv[:, :], var[:, :])
                nc.scalar.activation(rv[:, :], rv[:, :], AF.Sqrt)
                nc.vector.tensor_tensor(sc[:, :], rv[:, :], bng_sb[bn][ko][:, :],
                                        ALU.mult)
                # shf = beta' + mneg*scl
                nc.vector.scalar_tensor_tensor(out=sh[:, :], in0=mneg[:, :],
                                               scalar=sc[:, 0:1],
                                               in1=bnb_sb[bn][ko][:, :],
                                               op0=ALU.mult, op1=ALU.add)
                scl[bn][ko], shf[bn][ko] = sc, sh

            # row chunks for normalize/quantize pipelining (pad-row ranges)
            NCHUNKS = ((0, 9), (9, 18), (18, 26), (26, HP))
            # matching interior-row chunks for normalize
            NORM_CHUNKS = ((0, 8), (8, 17), (17, 25), (25, H))

            def norm_quant(bn, s, ko, eng, lo_eng, chunks=None):
                """chunk-interleaved: normalize rows in-place (eng), then
                hi=fp8(hn) on ACT and lo=fp8(hn-hi) on lo_eng for the same
                rows, so the first conv group starts after ~1/4 of the work.
                Quant chunks span the full pad width (borders are zero)."""
                h_pads = h1_pad if bn == 0 else h2_pad
                q = h1q if bn == 0 else h2q
                src = h_pads[s][ko]
                shift = bsk[s][ko] if bn == 0 else shf[bn][ko]
                sel = range(len(NCHUNKS)) if chunks is None else chunks
                with nc.allow_low_precision(reason="f32r bits / fp8 quant"):
                    for (r0, r1), (a, b) in [(NORM_CHUNKS[i], NCHUNKS[i])
                                             for i in sel]:
                        eng.tensor_scalar(
                            out=_interior(src, r0, r1 - r0),
                            in0=_interior(src, r0, r1 - r0),
                            scalar1=scl[bn][ko][:, 0:1],
                            scalar2=shift[:, 0:1],
                            op0=ALU.mult, op1=ALU.add)
                        cols = slice(ko * NPAD + a * WP, ko * NPAD + b * WP)
                        scols = slice(a * WP, b * WP)
                        nc.scalar.activation(q[s][0][:, cols], src[:, scols],
                                             AF.Identity)
                        lo_eng.tensor_tensor(q[s][1][:, cols], src[:, scols],
                                             q[s][0][:, cols], ALU.subtract)

            def make_bsk(s, ko, eng):
                b = pst.tile([128, 1], F32, name=f"bsk{s}{ko}")
                eng.tensor_tensor(b[:, :], shf[0][ko][:, :],
                                  te_sb[ko][:, s:s + 1], ALU.add)
                bsk[s][ko] = b

            # ---- DR conv emission ----
            def conv1_group(s, ko, half):
                """9 taps, ki-paired, 3 terms each; ko-major weight layout."""
                ps = psum(f"c1_{s}{ko}{half}")
                ps3 = ps[:, :].rearrange("p (r c) -> p r c", c=W)
                r0 = half * RH
                idx = 0
                for (wt, xt) in ((0, 0), (1, 0), (0, 1)):
                    for tap in range(9):
                        dy, dx = divmod(tap, 3)
                        woff = ((ko * 9 + tap) * 2) * 128
                        lhsT = _ap4(cw_sb[0][wt][:, :], woff,
                                    [[128, 2], [1, 128]])
                        rhs = _ap4(x8[s][xt][:, :], (r0 + dy) * WP + dx,
                                   [[NPAD, 2], [WP, RH], [1, W]])
                        nc.tensor.matmul(ps3, lhsT, rhs, start=(idx == 0),
                                         stop=(idx == 26), perf_mode=DR)
                        idx += 1
                return ps, ps3

            def conv_open(ci, qtiles, psums, ki, order):
                """open-phase: 4 within-ki tap pairs x 3 terms per group."""
                for (s, ko, half) in order:
                    ps3 = psums[(s, ko, half)][:, :].rearrange(
                        "p (r c) -> p r c", c=W)
                    r0 = half * RH
                    first = True
                    for (wt, xt) in ((0, 0), (1, 0), (0, 1)):
                        for (pi, (dya, dxa, dyb, dxb)) in enumerate(TAP_PAIRS):
                            woff = (((ki * 4 + pi) * 2) * KT + ko) * 128
                            ms = (dyb - dya) * WP + (dxb - dxa)
                            lhsT = _ap4(cw_sb[ci][wt][:, :], woff,
                                        [[KT * 128, 2], [1, 128]])
                            rhs = _ap4(qtiles[s][xt][:, :],
                                       ki * NPAD + (r0 + dya) * WP + dxa,
                                       [[ms, 2], [WP, RH], [1, W]])
                            nc.tensor.matmul(ps3, lhsT, rhs, start=first,
                                             stop=False, perf_mode=DR)
                            first = False

            def conv_close(ci, qtiles, psums, ki, order, bn=None, h_out=None,
                           epi3=None, descale=None):
                """close-phase: 4 within-ki pairs + cross-ki center pair."""
                for (s, ko, half) in order:
                    ps = psums[(s, ko, half)]
                    ps3 = ps[:, :].rearrange("p (r c) -> p r c", c=W)
                    r0 = half * RH
                    emits = []
                    for (pi, (dya, dxa, dyb, dxb)) in enumerate(TAP_PAIRS):
                        woff = (((ki * 4 + pi) * 2) * KT + ko) * 128
                        ms = (dyb - dya) * WP + (dxb - dxa)
                        emits.append((woff, KT * 128,
                                      ki * NPAD + (r0 + dya) * WP + dxa, ms))
                    # center tap paired across ki
                    emits.append(((32 + ko) * 128, KT * 128,
                                  (r0 + 1) * WP + 1, NPAD))
                    idx = 0
                    for (wt, xt) in ((0, 0), (1, 0), (0, 1)):
                        for (woff, wms, xoff, xms) in emits:
                            lhsT = _ap4(cw_sb[ci][wt][:, :], woff,
                                        [[wms, 2], [1, 128]])
                            rhs = _ap4(qtiles[s][xt][:, :], xoff,
                                       [[xms, 2], [WP, RH], [1, W]])
                            nc.tensor.matmul(ps3, lhsT, rhs, start=False,
                                             stop=(idx == 14), perf_mode=DR)
                            idx += 1
                    if epi3 is not None:
                        epi3(s, ko, half, ps)
                    else:
                        epilogue_bn(bn, h_out, s, ko, half, ps3, descale)

            # =============== conv1 (ko-major for per-ko stat sync) =========
            DS1 = 1.0 / (A_X * A_W)
            DS2 = 1.0 / (A_H * A_W)
            ccs = [None] * 4
            for ko in range(KT):
                for s in range(BL):
                    for half in range(2):
                        ps, ps3 = conv1_group(s, ko, half)
                        epilogue_bn(0, h1_pad, s, ko, half, ps3, DS1)
            ccs[0] = cc_launch_bn1()

            # time MLP on PE right after conv1 (fills part of the cc0 bubble)
            te1_sb = [pst.tile([128, BL], F32R, name=f"te1_{m}")
                      for m in range(KT)]
            te_sb = [pst.tile([128, BL], F32R, name=f"te_{m}")
                     for m in range(KT)]
            for mo in range(KT):
                ps = psum(f"mlp1_{mo}")
                for ki in range(KT):
                    nc.tensor.matmul(ps[:, 0:BL],
                                     w1t_sb[ki][:, mo * 128:(mo + 1) * 128],
                                     tt_sb[ki][:, :],
                                     start=(ki == 0), stop=(ki == KT - 1))
                nc.scalar.activation(te1_sb[mo][:, :], ps[:, 0:BL], AF.Relu,
                                     bias=bt1_sb[mo][:, :])
            for mo in range(KT):
                ps = psum(f"mlp2_{mo}")
                for ki in range(KT):
                    nc.tensor.matmul(ps[:, 0:BL],
                                     w2t_sb[ki][:, mo * 128:(mo + 1) * 128],
                                     te1_sb[ki][:, :],
                                     start=(ki == 0), stop=(ki == KT - 1))
                nc.scalar.activation(te_sb[mo][:, :], ps[:, 0:BL], AF.Relu,
                                     bias=bt2_sb[mo][:, :])

            # BN1 consts + normalize + quantize; s0 chain on DVE, s1 on
            # Pool.  Both ko blocks arrive in the one readback, so the two
            # kos' chunks interleave: conv2's ki-paired groups need both.
            cc_readback_bn1(ccs[0])
            for ko in range(KT):
                bn_consts(0, ko)
            make_bsk(0, 0, nc.vector)
            make_bsk(0, 1, nc.vector)
            make_bsk(1, 0, nc.gpsimd)
            make_bsk(1, 1, nc.gpsimd)
            for ci in range(len(NCHUNKS)):
                for ko in range(KT):
                    norm_quant(0, 0, ko, nc.vector, nc.gpsimd, chunks=[ci])
                for ko in range(KT):
                    norm_quant(0, 1, ko, nc.gpsimd, nc.vector, chunks=[ci])

            s_major = [(s, ko, half) for s in range(BL) for ko in range(KT)
                       for half in range(2)]

            def groups(ko=None, s=None):
                gs = [g for g in s_major
                      if (ko is None or g[1] == ko) and (s is None or g[0] == s)]
                # halves-0 first: they only need the top normalize/quantize
                # chunks, so the PE restarts sooner after a stat sync
                return sorted(gs, key=lambda g: (g[2], g[0], g[1]))

            # =============== conv2 (BN1 merged: straight through) ==========
            # Each group opens (ki0 pairs) and closes (ki1 pairs + cross-ki
            # center) back to back; ko0 groups run first so BN2-ko0's stat
            # sync launches at conv2's midpoint.
            psums2 = {g: psum(f"c2_{g[0]}{g[1]}{g[2]}") for g in s_major}
            warmup(warm1, psums2[s_major[0]])
            for ko in range(KT):
                for g in groups(ko=ko):
                    conv_open(1, h1q, psums2, ki=0, order=[g])
                    conv_close(1, h1q, psums2, ki=1, order=[g], bn=1,
                               h_out=h2_pad, descale=DS2)
                ccs[2 + ko] = cc_launch(1, ko)
            for ko in range(KT):
                cc_readback(ko, ccs[2 + ko])
                bn_consts(1, ko)
                norm_quant(1, 0, ko, nc.vector, nc.gpsimd)
                norm_quant(1, 1, ko, nc.gpsimd, nc.vector)

            # =============== conv3 (transform; bias, no relu) ==============
            y_sb = [[py.tile([128, N], dt, tag="y", name=f"y{s}{k}")
                     for k in range(KT)] for s in range(BL)]

            def epi3(s, ko, half, ps):
                with nc.allow_low_precision(reason="f32r==f32 bit layout"):
                    nc.vector.tensor_scalar(
                        out=y_sb[s][ko][:, half * 512:(half + 1) * 512],
                        in0=ps[:, :], scalar1=DS2,
                        scalar2=cb_sb[2][ko][:, :].bitcast(F32),
                        op0=ALU.mult, op1=ALU.add)

            # =============== attention (two-sample pipeline) ===============
            vt = [[None] * 8 for _ in range(BL)]
            q_sb = [None] * BL
            k_sb = [None] * BL
            ptiles = [[[None] * 8 for _ in range(2)] for _ in range(BL)]
            pacc = [[None] * 2 for _ in range(BL)]
            rcp = [[None] * 2 for _ in range(BL)]
            rb = [[None] * 2 for _ in range(BL)]
            ps_pd = [[None] * 2 for _ in range(BL)]
            ps_pb = [[None] * 2 for _ in range(BL)]
            res_t = [[None] * KT for _ in range(BL)]

            def pe_v(s):
                for nt in range(8):
                    ps = psum(f"v{s}{nt}")
                    pv = ps[:, 0:C]
                    for c2 in range(KT):
                        nc.tensor.matmul(pv, y_sb[s][c2][:, nt * 128:(nt + 1) * 128],
                                         wvt_sb[c2][:, :],
                                         start=(c2 == 0), stop=(c2 == KT - 1))
                    v = pat.tile([128, C], dt, tag="vt", bufs=16, name=f"vt{s}{nt}")
                    with nc.allow_low_precision(reason="f32r==f32 bit layout"):
                        nc.vector.tensor_tensor(v[:, :], pv, bvbc_sb[:, :], ALU.add)
                    vt[s][nt] = v

            def pe_qk(s, on_dve=False):
                q_sb[s] = pat.tile([CQ, N], dt, tag="q", bufs=2, name=f"q{s}")
                k_sb[s] = pat.tile([CQ, N], dt, tag="k", bufs=2, name=f"k{s}")
                for nh in range(2):
                    psq_ = psum(f"q{s}{nh}")
                    for c2 in range(KT):
                        nc.tensor.matmul(psq_[0:CQ, :], wqt_sb[c2][:, :],
                                         y_sb[s][c2][:, nh * 512:(nh + 1) * 512],
                                         start=(c2 == 0), stop=(c2 == KT - 1))
                    if on_dve:
                        with nc.allow_low_precision(reason="f32r bits"):
                            nc.vector.tensor_scalar(
                                out=q_sb[s][:, nh * 512:(nh + 1) * 512],
                                in0=psq_[0:CQ, :],
                                scalar1=bq_sb[:, :].bitcast(F32), scalar2=None,
                                op0=ALU.add)
                    else:
                        nc.scalar.activation(
                            q_sb[s][:, nh * 512:(nh + 1) * 512],
                            psq_[0:CQ, :], AF.Identity, bias=bq_sb[:, :])
                    psk_ = psum(f"k{s}{nh}")
                    for c2 in range(KT):
                        nc.tensor.matmul(psk_[0:CQ, :], wkt_sb[c2][:, :],
                                         y_sb[s][c2][:, nh * 512:(nh + 1) * 512],
                                         start=(c2 == 0), stop=(c2 == KT - 1))
                    if on_dve:
                        with nc.allow_low_precision(reason="f32r bits"):
                            nc.vector.tensor_scalar(
                                out=k_sb[s][:, nh * 512:(nh + 1) * 512],
                                in0=psk_[0:CQ, :],
                                scalar1=bk_sb[:, :].bitcast(F32), scalar2=None,
                                op0=ALU.add)
                    else:
                        nc.scalar.activation(
                            k_sb[s][:, nh * 512:(nh + 1) * 512],
                            psk_[0:CQ, :], AF.Identity, bias=bk_sb[:, :])

            def pe_s(s, nh):
                for mt in range(8):
                    ps = psum(f"s{s}{nh}{mt}")
                    nc.tensor.matmul(ps[:, :], k_sb[s][:, mt * 128:(mt + 1) * 128],
                                     q_sb[s][:, nh * 512:(nh + 1) * 512],
                                     start=True, stop=True)
                    p = pat.tile([128, 512], dt, tag="P", bufs=9,
                                 name=f"P{s}{nh}{mt}")
                    nc.scalar.activation(p[:, :], ps[:, :], AF.Exp)
                    ptiles[s][nh][mt] = p

            _pacca = {}

            def pool_pacc(s, nh, split=False):
                pt = ptiles[s][nh]
                tag = "pacca" if split else "pacc"
                pa = pat.tile([128, 512], dt, tag=tag, bufs=2,
                              name=f"pacca{s}{nh}")
                if split:
                    _pacca[(s, nh)] = pa
                else:
                    pacc[s][nh] = pa
                hi = 4 if split else 8
                with nc.allow_low_precision(reason="f32r==f32 bit layout"):
                    nc.gpsimd.tensor_tensor(pa[:, :], pt[0][:, :],
                                            pt[1][:, :], ALU.add)
                    for mt in range(2, hi):
                        nc.gpsimd.tensor_tensor(pa[:, :], pa[:, :],
                                                pt[mt][:, :], ALU.add)

            def dve_pacc(s, nh):
                pt = ptiles[s][nh]
                pa = pat.tile([128, 512], dt, tag="pacc", bufs=2,
                              name=f"paccb{s}{nh}")
                pacc[s][nh] = pa
                # fold pacca in BEFORE the last exp tile so only one add
                # remains on the post-last-exp critical path
                with nc.allow_low_precision(reason="f32r==f32 bit layout"):
                    nc.vector.tensor_tensor(pa[:, :], pt[4][:, :],
                                            pt[5][:, :], ALU.add)
                    nc.vector.tensor_tensor(pa[:, :], pa[:, :],
                                            pt[6][:, :], ALU.add)
                    nc.vector.tensor_tensor(pa[:, :], pa[:, :],
                                            _pacca[(s, nh)][:, :], ALU.add)
                    nc.vector.tensor_tensor(pa[:, :], pa[:, :],
                                            pt[7][:, :], ALU.add)

            _vp_psum = {}
            _vp_sbuf = {}

            def pe_vp(s, nh):
                for c2 in range(KT):
                    pr = psum(f"vp{s}{nh}{c2}")
                    for mt in range(8):
                        nc.tensor.matmul(pr[:, :],
                                         vt[s][mt][:, c2 * 128:(c2 + 1) * 128],
                                         ptiles[s][nh][mt][:, :],
                                         start=(mt == 0), stop=(mt == 7))
                    _vp_psum[(s, nh, c2)] = pr

            def act_vpcopy(s, nh):
                for c2 in range(KT):
                    t_ = pat.tile([128, 512], F32, tag="vps", bufs=2,
                                  name=f"vpsa{s}{nh}{c2}")
                    nc.scalar.activation(t_[:, :], _vp_psum[(s, nh, c2)][:, :],
                                         AF.Identity)
                    _vp_sbuf[(s, nh, c2)] = t_

            def pool_vpcopy(s, nh):
                for c2 in range(KT):
                    t_ = pat.tile([128, 512], F32, tag="vps", bufs=2,
                                  name=f"vps{s}{nh}{c2}")
                    nc.vector.tensor_copy(t_[:, :], _vp_psum[(s, nh, c2)][:, :])
                    _vp_sbuf[(s, nh, c2)] = t_

            def pe_pd(s, nh):
                pd = psum(f"pd{s}{nh}")
                nc.tensor.matmul(pd[0:1, :], ones_col[:, :], pacc[s][nh][:, :],
                                 start=True, stop=True)
                ps_pd[s][nh] = pd

            def dve_rcp(s, nh):
                r = pat.tile([1, 512], dt, tag="rcp", bufs=2, name=f"rcp{s}{nh}")
                with nc.allow_low_precision(reason="f32r==f32 bit layout"):
                    nc.vector.reciprocal(r[:, :], ps_pd[s][nh][0:1, :])
                rcp[s][nh] = r

            def pe_pb(s, nh):
                pb = psum(f"pb{s}{nh}")
                nc.tensor.matmul(pb[:, :], ones_row[:, :], rcp[s][nh][:, :],
                                 start=True, stop=True)
                ps_pb[s][nh] = pb

            def pool_rb(s, nh, on_act=False):
                r = pat.tile([128, 512], F32, tag="rb", bufs=2, name=f"rb{s}{nh}")
                if on_act:
                    nc.scalar.activation(r[:, :], ps_pb[s][nh][:, :], AF.Identity)
                else:
                    nc.vector.tensor_copy(r[:, :], ps_pb[s][nh][:, :])
                rb[s][nh] = r

            def dve_res(s, nh, direct_rb=False):
                rbs = ps_pb[s][nh] if direct_rb else rb[s][nh]
                for c2 in range(KT):
                    if res_t[s][c2] is None:
                        res_t[s][c2] = pat.tile([128, N], F32R, tag="res", bufs=2,
                                                name=f"res{s}{c2}")
                    rs = res_t[s][c2][:, nh * 512:(nh + 1) * 512]
                    pr = _vp_sbuf[(s, nh, c2)]
                    with nc.allow_low_precision(reason="f32r==f32 bit layout"):
                        nc.vector.tensor_tensor(rs, pr[:, :], rbs[:, :],
                                                ALU.mult)
                        nc.vector.tensor_tensor(
                            rs, rs, y_sb[s][c2][:, nh * 512:(nh + 1) * 512],
                            ALU.add)

            def dma_res(s, nh):
                for c2 in range(KT):
                    nc.sync.dma_start(
                        out=out_d[s, c2, :, nh * 512:(nh + 1) * 512],
                        in_=res_t[s][c2][:, nh * 512:(nh + 1) * 512])

            # conv3: ki0 open gated on BN2-ko0; close ko0 groups first (same
            # gate), ko1 closes gated on BN2-ko1; s0's y completes right
            # after the (0,1,*) closes so attention starts early.
            psums3 = {g: psum(f"c3_{g[0]}{g[1]}{g[2]}") for g in s_major}
            warmup(warm3, psums3[s_major[0]])
            conv_open(2, h2q, psums3, ki=0, order=groups(ko=0))
            conv_open(2, h2q, psums3, ki=0, order=[(0, 1, 0), (0, 1, 1)])
            warmup(warm4, psums3[(1, 1, 0)])
            conv_open(2, h2q, psums3, ki=0, order=[(1, 1, 0), (1, 1, 1)])
            conv_close(2, h2q, psums3, ki=1, order=groups(ko=0), epi3=epi3)
            conv_close(2, h2q, psums3, ki=1, order=[(0, 1, 0), (0, 1, 1)],
                       epi3=epi3)
            # sample-0 attention starts while sample-1 still closes: the s0
            # exp stream (ACT) runs under the close11 matmuls
            pe_qk(0)
            pe_s(0, 0)
            pe_v(0)
            conv_close(2, h2q, psums3, ki=1, order=[(1, 1, 0), (1, 1, 1)],
                       epi3=epi3)
            pe_s(0, 1)
            pool_pacc(0, 0)
            pe_vp(0, 0)
            pool_vpcopy(0, 0)
            pe_pd(0, 0)
            pe_v(1)
            pe_qk(1)
            dve_rcp(0, 0)
            pe_pb(0, 0)
            pool_rb(0, 0)
            pool_pacc(0, 1)
            pe_vp(0, 1)
            pool_vpcopy(0, 1)
            dve_res(0, 0)
            dma_res(0, 0)
            pe_pd(0, 1)
            pe_s(1, 0)
            dve_rcp(0, 1)
            pe_pb(0, 1)
            pool_rb(0, 1)
            pe_s(1, 1)
            dve_res(0, 1)
            dma_res(0, 1)
            pool_pacc(1, 0)
            pe_vp(1, 0)
            pool_vpcopy(1, 0)
            pe_pd(1, 0)
            dve_rcp(1, 0)
            pe_pb(1, 0)
            pool_rb(1, 0)
            pool_pacc(1, 1, split=True)
            dve_pacc(1, 1)
            pe_vp(1, 1)
            act_vpcopy(1, 1)
            dve_res(1, 0)
            dma_res(1, 0)
            pe_pd(1, 1)
            dve_rcp(1, 1)
            pe_pb(1, 1)
            dve_res(1, 1, direct_rb=True)
            dma_res(1, 1)

    _split_packed_waits(nc)
    return nc


FP8NP = ml_dtypes.float8_e4m3


def _fp8_hilo(a):
    """split float array into e4m3 hi + lo (both returned as fp8)."""
    hi = a.astype(FP8NP)
    lo = (a - hi.astype(np.float32)).astype(FP8NP)
    return hi, lo


def _prep_inputs(inputs):
    """host-side reshape/transpose/quantize; returns per_core input maps"""
    f32 = np.float32
    x = np.asarray(inputs["x"], f32)
    t = np.asarray(inputs["t"], f32)

    def conv1_w(w):
        # ko-major cross-ki layout: col block j = (ko*9+tap)*2 + ki
        w6 = np.asarray(w, f32).reshape(KT, 128, KT, 128, 3, 3)  # ko,o,ki,i,dy,dx
        arr = np.zeros((128, CWC), f32)
        for ko in range(KT):
            for tap in range(9):
                dy, dx = divmod(tap, 3)
                for ki in range(KT):
                    j = (ko * 9 + tap) * 2 + ki
                    arr[:, j * 128:(j + 1) * 128] = w6[ko, :, ki, :, dy, dx].T
        return arr

    def conv23_w(w):
        # pair layout: A-section j = ((ki*4+pair)*2+member)*KT+ko;
        # B-section (center, ki-paired) j = 32 + ki*KT + ko
        w6 = np.asarray(w, f32).reshape(KT, 128, KT, 128, 3, 3)
        arr = np.zeros((128, CWC), f32)
        for ki in range(KT):
            for pi, (dya, dxa, dyb, dxb) in enumerate(TAP_PAIRS):
                for m, (dy, dx) in enumerate(((dya, dxa), (dyb, dxb))):
                    for ko in range(KT):
                        j = ((ki * 4 + pi) * 2 + m) * KT + ko
                        arr[:, j * 128:(j + 1) * 128] = w6[ko, :, ki, :, dy, dx].T
        for ki in range(KT):
            for ko in range(KT):
                j = 32 + ki * KT + ko
                arr[:, j * 128:(j + 1) * 128] = w6[ko, :, ki, :, 1, 1].T
        return arr

    cw = np.zeros((3, 2, 128, CWC), FP8NP)
    for ci, (lay, key) in enumerate(((conv1_w, "w_c1"), (conv23_w, "w_c2"),
                                     (conv23_w, "w_tr"))):
        hi, lo = _fp8_hilo(lay(inputs[key]) * A_W)
        cw[ci, 0], cw[ci, 1] = hi, lo

    w1t = np.ascontiguousarray(np.asarray(inputs["w_t1"], f32).T.reshape(KT, 128, T))
    w2t = np.ascontiguousarray(
        (np.asarray(inputs["w_t2"], f32) * A_H).T.reshape(KT, 128, C))
    consts = np.zeros((128, 22), f32)
    for ci, k2 in enumerate(("b_c1", "b_c2", "b_tr")):
        consts[:, ci * KT:(ci + 1) * KT] = np.asarray(inputs[k2], f32).reshape(KT, 128).T
    for i, (gk, bk2) in enumerate((("bn1_g", "bn1_b"), ("bn2_g", "bn2_b"))):
        consts[:, 6 + i * KT:6 + (i + 1) * KT] = \
            (np.asarray(inputs[gk], f32) * A_H).reshape(KT, 128).T
        consts[:, 10 + i * KT:10 + (i + 1) * KT] = \
            (np.asarray(inputs[bk2], f32) * A_H).reshape(KT, 128).T
    consts[:, 14:16] = np.asarray(inputs["b_t1"], f32).reshape(KT, 128).T
    consts[:, 16:18] = (np.asarray(inputs["b_t2"], f32) * A_H).reshape(KT, 128).T
    wqt = np.ascontiguousarray(np.asarray(inputs["wq"], f32).T.reshape(KT, 128, CQ))
    wkt = np.ascontiguousarray(np.asarray(inputs["wk"], f32).T.reshape(KT, 128, CQ))
    gam_v = np.asarray(inputs["gamma"], f32).reshape(())
    wvt = np.ascontiguousarray(
        (np.asarray(inputs["wv"], f32) * gam_v).T.reshape(KT, 128, C))
    bq = np.asarray(inputs["bq"], f32).reshape(CQ, 1)
    bk = np.asarray(inputs["bk"], f32).reshape(CQ, 1)
    bvbc = np.ascontiguousarray(
        np.tile((np.asarray(inputs["bv"], f32) * gam_v).reshape(1, C), (128, 1)))

    # x: scaled, padded, hi/lo fp8, ki blocks side by side
    xs = x.reshape(B, KT, 128, H, W) * A_X
    xp = np.zeros((B, 2, 128, KT, HP, WP), f32)
    hi, lo = _fp8_hilo(xs)
    xp[:, 0, :, :, 1:1 + H, 1:1 + W] = hi.astype(f32).transpose(0, 2, 1, 3, 4)
    xp[:, 1, :, :, 1:1 + H, 1:1 + W] = lo.astype(f32).transpose(0, 2, 1, 3, 4)
    xp8 = xp.reshape(B, 2, 128, KT * NPAD).astype(FP8NP)
    ttr = np.ascontiguousarray(t.T.reshape(KT, 128, B))

    shared = dict(cw=cw, w1t=w1t, w2t=w2t,
                  wqt=wqt, wkt=wkt, wvt=wvt, bq=bq, bk=bk, bvbc=bvbc)
    per_core = []
    for c in range(NCORES):
        m = dict(shared)
        m["xp"] = np.ascontiguousarray(xp8[c * BL:(c + 1) * BL])
        cc_consts = consts.copy()
        for k in range(KT):
            cc_consts[:, 18 + k * BL:18 + (k + 1) * BL] = \
                ttr[k, :, c * BL:(c + 1) * BL]
        m["consts"] = cc_consts
        per_core.append(m)
    return per_core


def _unshard(results):
    out = np.empty((B, C, H, W), np.float32)
    for c in range(NCORES):
        o = results[c]["out"].reshape(BL, KT, 128, H, W)
        for s in range(BL):
            out[c * BL + s] = o[s].reshape(C, H, W)
    return out


_cache = {}


def kernel(**inputs) -> np.ndarray:
    key = "nc"
    if key not in _cache:
        _cache[key] = build()
    nc = _cache[key]
    per_core = _prep_inputs(inputs)
    try:
        res = run_bass_kernel_spmd(nc, per_core, core_ids=list(range(NCORES)))
    except Exception:
        # transient NRT_EXEC_UNIT_UNRECOVERABLE errors recover on re-execute
        res = run_bass_kernel_spmd(nc, per_core, core_ids=list(range(NCORES)))
    return _unshard(res.results)
